# revision 1
# baseline (speedup 1.0000x reference)
"""Trainium2 Bass kernel for nn_MultiHeadSelfAttention_62646392979761.

Math (per the buggy-einsum reference): per position s, heads attend to heads:
  Q,K,V = x@W{q,k,v}.T + b  (N,S,H,D);  scores[s] = Q[s]K[s]^T/8 (16x16);
  A = softmax_j;  AV[s] = A[s]V[s];  out2 = scrambled reshape (16-position
  groups of one head per output row);  final = out2@Wo.T + bo.

Sharding: 8 cores x 2048 rows of the flattened (16384, 1024) x. Attention is
position-local; the scramble groups 16 consecutive positions, which never
cross a 2048-row shard. Zero cross-core communication.

Per-core device pipeline (16 tiles x 128 positions), all layouts validated
against the reference in a numpy simulator:
  1. QKV projections: PE matmuls, stationary xT e-chunks, moving fused
     [Wq/8|Wk|Wv]^T bf16; DVE bias-add evac -> QN/KN/VN bf16.
  2. xbar DMA transposes -> QT/KT chunks [(i2,d), slot].
  3. QBLK (masked block-diag pairs) + KBLK via 4 fused strided DVE copies
     each; structural zeros memset once.
  4. Scores: 64 pair-packed matmuls (k=128, m=32 col-rotated, n=16) ->
     SCO psum [(u,p,i), (j,gg)].
  5. ACT exp -> E bf16; DVE j-reduce -> Z; reciprocal -> Zr.
  6. E -> DRAM bounce -> ABLK [(p,j), (p,i,u,gg)] (masked, 32B-run DMA);
     VN -> DRAM bounce -> VTHP [(p,j), (g,d)] (128B-run DMA).
  7. AV: 64 pair matmuls (k=32, m=32 col-rotated, n=64) -> ANAT psum
     [(u,p,i), (gg,d)]; evac * Zr (per-gg tensor_scalar) -> bf16.
  8. xbar ANAT slices -> AVB2 [(b,d), (t,g',u,p,i)] accumulated all tiles.
  9. Final projection: host-permuted WoT chunks stationary, AVB2 strided
     rhs, 8-chunk psum accumulation, + bo -> outT (1024, 2048) f32.
Host: pre-permutes x rows (even/odd within tile), pre-transposes weights,
post-scatters finalT columns to (n, s_out) rows.
"""

import math
import numpy as np
import ml_dtypes

TILES = 16
ROWS = 2048
NB, SB, EB, HB, DB = 4, 4096, 1024, 16, 64

_CACHE = {}


def _split_waits_json(bir_bytes):
    """This env's walrus accepts only ONE embedded sync-wait per TPB
    instruction (NEURON_ISA_TPB_EVENTS has a single wait slot) but Tile emits
    several. Split excess on_wait entries onto standalone EventSemaphore
    instructions inserted just before, on the same engine — semantically
    identical on in-order engine queues."""
    import json
    d = json.loads(bir_bytes)
    for fn in d.get('functions', []):
        for bb in (fn.get('basic_blocks') or fn.get('blocks') or []):
            out = []
            for inst in bb.get('instructions', []):
                si = inst.get('sync_info')
                w = (si or {}).get('on_wait') or []
                if len(w) > 1:
                    for k, extra in enumerate(w[:-1]):
                        out.append({
                            'debug': inst.get('debug', 0),
                            'engine': inst['engine'],
                            'ins': [], 'outs': [],
                            'name': f"{inst['name']}-sw{k}",
                            'opcode': 'EventSemaphore',
                            'sync_info': {'on_wait': [extra], 'on_update': []},
                        })
                    si['on_wait'] = [w[-1]]
                out.append(inst)
            bb['instructions'] = out
    return json.dumps(d).encode()


def _install_birpatch():
    import concourse.bass_utils as bu
    import concourse.bass2jax as b2j
    if getattr(bu.compile_bir_kernel, '_waitsplit', False):
        return
    orig = bu.compile_bir_kernel

    def patched(bir_json, tmpdir, neff_name="file.neff"):
        return orig(_split_waits_json(bir_json), tmpdir, neff_name)

    patched._waitsplit = True
    bu.compile_bir_kernel = patched
    b2j.compile_bir_kernel = patched


def _build_bass():
    import concourse.bass as bass
    import concourse.tile as tile
    from concourse import mybir

    bf16 = mybir.dt.bfloat16
    f32 = mybir.dt.float32
    AF = mybir.ActivationFunctionType
    ALU = mybir.AluOpType
    AX = mybir.AxisListType

    nc = bass.Bass(trn_type="TRN2")
    xt_d = nc.declare_dram_parameter("xt", [1024, ROWS], bf16, isOutput=False)
    wqkv_d = nc.declare_dram_parameter("wqkv", [1024, 3072], bf16, isOutput=False)
    bias_d = nc.declare_dram_parameter("bqkv", [128, 3072], bf16, isOutput=False)
    wo_d = nc.declare_dram_parameter("wo", [1024, 1024], bf16, isOutput=False)
    bo_d = nc.declare_dram_parameter("bo", [1024, 1], f32, isOutput=False)
    out_d = nc.declare_dram_parameter("out", [1024, ROWS], f32, isOutput=True)

    from contextlib import ExitStack
    with ExitStack() as ctx:
        tc = ctx.enter_context(tile.TileContext(nc))
        const = ctx.enter_context(tc.tile_pool(name="const", bufs=1))
        work = ctx.enter_context(tc.tile_pool(name="work", bufs=2))
        psq = ctx.enter_context(tc.tile_pool(name="psq", bufs=2, space="PSUM"))
        pss = ctx.enter_context(tc.tile_pool(name="pss", bufs=1, space="PSUM"))
        drp = ctx.enter_context(tc.tile_pool(name="drp", bufs=2, space="DRAM"))

        # ---- persistent tensors ----
        wq_sb = const.tile([128, 8 * 3072], bf16, tag="wq")
        wo_sb = const.tile([128, 8 * 1024], bf16, tag="wo")
        bias_sb = const.tile([128, 3072], bf16, tag="bias")
        bo_sb = const.tile([128, 8], f32, tag="bo")
        avb2 = const.tile([128, TILES * 1024], bf16, tag="avb2")

        # weight loads: wqkv chunk ec -> cols [3072*ec : +3072]
        src = xt_d  # placeholder to appease linters
        nc.sync.dma_start(
            wq_sb[:].rearrange("p (c f) -> p c f", c=8),
            wqkv_d[:].rearrange("(c p) f -> p c f", c=8),
        )
        nc.sync.dma_start(
            wo_sb[:].rearrange("p (q f) -> p q f", q=8),
            wo_d[:].rearrange("(q p) f -> p q f", q=8),
        )
        nc.sync.dma_start(bias_sb[:], bias_d[:])
        nc.sync.dma_start(
            bo_sb[:],
            bo_d[:].rearrange("(c p) one -> p c one", c=8)[:, :, 0],
        )

        for t in range(TILES):
            # ---- load xT tile: xt_sb[:, 128c:+128] = xt_d[128c:+128, 128t:+128]
            xt_sb = work.tile([128, 1024], bf16, tag="xt")
            nc.sync.dma_start(
                xt_sb[:].rearrange("p (c s) -> p c s", c=8),
                xt_d[:].rearrange("(c p) s -> p c s", c=8)[:, :, 128 * t:128 * (t + 1)],
            )
            # ---- projections (Q, K, V sequentially through 2-bank psum) ----
            qn = work.tile([128, 1024], bf16, tag="qn")
            kn = work.tile([128, 1024], bf16, tag="kn")
            vn = work.tile([128, 1024], bf16, tag="vn")
            for w, dst in enumerate((qn, kn, vn)):
                psp = psq.tile([128, 1024], f32, tag="psp", name="psp")
                for ec in range(8):
                    lhsT = xt_sb[:, 128 * ec:128 * (ec + 1)]
                    for half in range(2):
                        rhs = wq_sb[:, 3072 * ec + 1024 * w + 512 * half:
                                    3072 * ec + 1024 * w + 512 * (half + 1)]
                        nc.tensor.matmul(
                            psp[:, 512 * half:512 * (half + 1)], lhsT, rhs,
                            start=(ec == 0), stop=(ec == 7))
                nc.vector.tensor_add(dst[:], psp[:], bias_sb[:, 1024 * w:1024 * (w + 1)])
            # ---- xbar transposes Q,K ----
            qt = work.tile([128, 1024], bf16, tag="qt")
            kt = work.tile([128, 1024], bf16, tag="kt")
            for c in range(8):
                nc.sync.dma_start_transpose(qt[:, 128 * c:128 * (c + 1)], qn[:, 128 * c:128 * (c + 1)])
                nc.sync.dma_start_transpose(kt[:, 128 * c:128 * (c + 1)], kn[:, 128 * c:128 * (c + 1)])
            # ---- QBLK / KBLK fused copies (double-buffered; re-zero masked) ----
            qblk = work.tile([128, 2048], bf16, tag="qblk", name="qblk")
            kblk = work.tile([128, 1024], bf16, tag="kblk", name="kblk")
            nc.vector.memset(qblk[:], 0.0)
            for p in range(2):
                for i2 in range(2):
                    srcq = qt[:][64 * i2:64 * i2 + 64, :].rearrange(
                        "p (c s) -> p c s", c=8)[:, :, 64 * p:64 * p + 64]
                    dstq = qblk[:][64 * p:64 * p + 64, 1024 * p:1024 * p + 1024].rearrange(
                        "p (c a g) -> p c a g", c=8, a=2)[:, :, i2, :]
                    nc.vector.tensor_copy(dstq, srcq)
                    srck = kt[:][64 * i2:64 * i2 + 64, :].rearrange(
                        "p (c s) -> p c s", c=8)[:, :, 64 * p:64 * p + 64]
                    dstk = kblk[:][64 * p:64 * p + 64, :].rearrange(
                        "p (c a g) -> p c a g", c=8, a=2)[:, :, i2, :]
                    nc.vector.tensor_copy(dstk, srck)
            # ---- scores: 64 pair matmuls ----
            sco = pss.tile([128, 256], f32, tag="sco")
            qv = qblk[:].rearrange("p (a i g) -> p a i g", a=2, i=16)
            kv = kblk[:].rearrange("p (j g) -> p j g", j=16)
            scov = sco[:].rearrange("p (j gg) -> p j gg", j=16)
            for g in range(64):
                u, gg = g % 4, g // 4
                nc.tensor.matmul(
                    scov[32 * u:32 * u + 32, :, gg],
                    qv[:, :, :, g], kv[:, :, g],
                    start=True, stop=True, tile_position=(0, 32 * u))
            # ---- softmax pieces ----
            ex = work.tile([128, 256], bf16, tag="ex")
            nc.scalar.activation(ex[:], sco[:], func=AF.Exp)
            z = work.tile([128, 16], f32, tag="z")
            zr = work.tile([128, 16], f32, tag="zr")
            nc.vector.tensor_reduce(
                z[:], ex[:].rearrange("p (j gg) -> p gg j", j=16),
                axis=AX.X, op=ALU.add)
            nc.vector.reciprocal(zr[:], z[:])
            # ---- bounce E -> ABLK ----
            ablk = work.tile([32, 2048], bf16, tag="ablk", name="ablk")
            vthp = work.tile([32, 4096], bf16, tag="vthp", name="vthp")
            nc.vector.memset(ablk[:], 0.0)
            exd = drp.tile([128, 256], bf16, tag="exd")
            nc.sync.dma_start(exd[:], ex[:])
            for p in range(2):
                for u in range(4):
                    dst = ablk[:][16 * p:16 * p + 16, 1024 * p:1024 * p + 1024].rearrange(
                        "P (i u gg) -> P i u gg", i=16, u=4)[:, :, u, :]
                    srce = exd[:].rearrange(
                        "(u a i) (j gg) -> u a j i gg", u=4, a=2, i=16, j=16)[u, p]
                    nc.sync.dma_start(dst, srce)
            # ---- bounce VN -> VTHP ----
            vnd = drp.tile([128, 1024], bf16, tag="vnd")
            nc.sync.dma_start(vnd[:], vn[:])
            for p in range(2):
                nc.sync.dma_start(
                    vthp[:][16 * p:16 * p + 16, :].rearrange("P (g d) -> P g d", g=64),
                    vnd[:].rearrange("(a g) (j d) -> a j g d", a=2, j=16)[p])
            # ---- AV: 64 pair matmuls ----
            anat = pss.tile([128, 1024], f32, tag="anat")
            av = ablk[:].rearrange("P (a i u gg) -> P a i u gg", a=2, i=16, u=4)
            vv = vthp[:].rearrange("P (g d) -> P g d", g=64)
            anv = anat[:].rearrange("p (gg d) -> p gg d", gg=16)
            for g in range(64):
                u, gg = g % 4, g // 4
                nc.tensor.matmul(
                    anv[32 * u:32 * u + 32, gg, :],
                    av[:, :, :, u, gg], vv[:, g, :],
                    start=True, stop=True, tile_position=(0, 32 * u))
            # ---- normalize by 1/Z and evac ----
            anat_sb = work.tile([128, 1024], bf16, tag="anat_sb")
            asv = anat_sb[:].rearrange("p (gg d) -> p gg d", gg=16)
            for gg in range(16):
                nc.vector.tensor_scalar_mul(asv[:, gg, :], anv[:, gg, :], zr[:, gg:gg + 1])
            # ---- xbar ANAT -> AVB2 ----
            for c4 in range(8):
                nc.sync.dma_start_transpose(
                    avb2[:, 1024 * t + 128 * c4:1024 * t + 128 * (c4 + 1)],
                    anat_sb[:, 128 * c4:128 * (c4 + 1)])

        # ---- final projection ----
        avv = avb2[:].rearrange("p (t c u a i) -> p t c u a i", t=TILES, c=8, u=4, a=2)
        for fc in range(8):
            for tg in range(4):
                psf = pss.tile([128, 512], f32, tag="psf")
                for q in range(8):
                    u, pq = q // 2, q % 2
                    nc.tensor.matmul(
                        psf[:], wo_sb[:, 1024 * q + 128 * fc:1024 * q + 128 * (fc + 1)],
                        avv[:, 4 * tg:4 * (tg + 1), :, u, pq, :],
                        start=(q == 0), stop=(q == 7))
                osb = work.tile([128, 512], f32, tag="osb")
                nc.vector.tensor_scalar_add(osb[:], psf[:], bo_sb[:, fc:fc + 1])
                nc.sync.dma_start(out_d[128 * fc:128 * (fc + 1), 512 * tg:512 * (tg + 1)], osb[:])
    return nc


def _host_prep(x, Wq, bq, Wk, bk, Wv, bv, Wo, bo):
    """Returns per-core input maps + post-scatter metadata."""
    xf = np.ascontiguousarray(x.reshape(NB * SB, EB))
    # slot permutation within each 128-tile: col 128t + 64p + g <- row 128t+2g+p
    idx = np.arange(ROWS)
    t, r = idx // 128, idx % 128
    p, g = r // 64, r % 64
    perm = 128 * t + 2 * g + p
    Wqs, bqs = Wq / 8.0, bq / 8.0
    WQKVT = np.concatenate([Wqs.T, Wk.T, Wv.T], axis=1).astype(ml_dtypes.bfloat16)
    BQKV = np.tile(np.concatenate([bqs, bk, bv])[None, :], (128, 1)).astype(ml_dtypes.bfloat16)
    WoTP = np.zeros((1024, 1024), np.float32)
    for u in range(4):
        for pp in range(2):
            q = 2 * u + pp
            for b in range(2):
                m = 8 * b + 2 * u + pp
                WoTP[q * 128 + b * 64:q * 128 + (b + 1) * 64, :] = Wo[:, m * 64:(m + 1) * 64].T
    WoTP = WoTP.astype(ml_dtypes.bfloat16)
    boT = bo.reshape(1024, 1).astype(np.float32)
    in_maps = []
    for core in range(8):
        n, s0 = core // 2, (core % 2) * ROWS
        xs = xf[n * SB + s0:n * SB + s0 + ROWS][perm]
        xT = np.ascontiguousarray(xs.T).astype(ml_dtypes.bfloat16)
        in_maps.append({"xt": xT, "wqkv": WQKVT, "bqkv": BQKV, "wo": WoTP, "bo": boT})
    return in_maps


def kernel(x, Wq, bq, Wk, bk, Wv, bv, Wo, bo):
    _install_birpatch()
    from concourse.bass_utils import run_bass_kernel_spmd

    if "nc" not in _CACHE:
        _CACHE["nc"] = _build_bass()
    nc = _CACHE["nc"]
    in_maps = _host_prep(np.asarray(x, np.float32), *[np.asarray(a, np.float32)
                         for a in (Wq, bq, Wk, bk, Wv, bv, Wo, bo)])
    res = run_bass_kernel_spmd(nc, in_maps, list(range(8)))
    out = np.zeros((NB, SB, EB), np.float32)
    # col t*128 + c4*16 + h -> row s_out = h*256 + (s0/16 + 8t + c4)
    tt = np.arange(ROWS)
    ct, cc4, ch = tt // 128, (tt // 16) % 8, tt % 16
    for core in range(8):
        n, s0 = core // 2, (core % 2) * ROWS
        fT = np.asarray(res.results[core]["out"])  # (1024, 2048)
        rows = ch * 256 + (s0 // 16 + 8 * ct + cc4)
        out[n, rows, :] = fT.T
    return out



# revision 3
# speedup vs baseline: 66.7964x; 66.7964x over previous
"""Trainium2 Bass kernel for nn_MultiHeadSelfAttention_62646392979761.

Math (per the buggy-einsum reference): per position s, heads attend to heads:
  Q,K,V = x@W{q,k,v}.T + b  (N,S,H,D);  scores[s] = Q[s]K[s]^T/8 (16x16);
  A = softmax_j;  AV[s] = A[s]V[s];  out2 = scrambled reshape (16-position
  groups of one head per output row);  final = out2@Wo.T + bo.

Sharding: 8 cores x 2048 rows of the flattened (16384, 1024) x. Attention is
position-local; the scramble groups 16 consecutive positions, which never
cross a 2048-row shard. Zero cross-core communication.

Per-core device pipeline (16 tiles x 128 positions), all layouts validated
against the reference in a numpy simulator:
  1. QKV projections: PE matmuls, stationary xT e-chunks, moving fused
     [Wq/8|Wk|Wv]^T bf16; DVE bias-add evac -> QN/KN/VN bf16.
  2. xbar DMA transposes -> QT/KT chunks [(i2,d), slot].
  3. QBLK (masked block-diag pairs) + KBLK via 4 fused strided DVE copies
     each; structural zeros memset once.
  4. Scores: 64 pair-packed matmuls (k=128, m=32 col-rotated, n=16) ->
     SCO psum [(u,p,i), (j,gg)].
  5. ACT exp -> E bf16; DVE j-reduce -> Z; reciprocal -> Zr.
  6. E -> DRAM bounce -> ABLK [(p,j), (p,i,u,gg)] (masked, 32B-run DMA);
     VN -> DRAM bounce -> VTHP [(p,j), (g,d)] (128B-run DMA).
  7. AV: 64 pair matmuls (k=32, m=32 col-rotated, n=64) -> ANAT psum
     [(u,p,i), (gg,d)]; evac * Zr (per-gg tensor_scalar) -> bf16.
  8. xbar ANAT slices -> AVB2 [(b,d), (t,g',u,p,i)] accumulated all tiles.
  9. Final projection: host-permuted WoT chunks stationary, AVB2 strided
     rhs, 8-chunk psum accumulation, + bo -> outT (1024, 2048) f32.
Host: pre-permutes x rows (even/odd within tile), pre-transposes weights,
post-scatters finalT columns to (n, s_out) rows.
"""

import math
import numpy as np
import ml_dtypes

TILES = 16
ROWS = 2048
NB, SB, EB, HB, DB = 4, 4096, 1024, 16, 64

_CACHE = {}


def _split_waits_json(bir_bytes):
    """This env's walrus accepts only ONE embedded sync-wait per TPB
    instruction (NEURON_ISA_TPB_EVENTS has a single wait slot) but Tile emits
    several. Split excess on_wait entries onto standalone EventSemaphore
    instructions inserted just before, on the same engine — semantically
    identical on in-order engine queues."""
    import json
    d = json.loads(bir_bytes)
    for fn in d.get('functions', []):
        for bb in (fn.get('basic_blocks') or fn.get('blocks') or []):
            out = []
            for inst in bb.get('instructions', []):
                si = inst.get('sync_info')
                w = (si or {}).get('on_wait') or []
                if len(w) > 1:
                    for k, extra in enumerate(w[:-1]):
                        out.append({
                            'debug': inst.get('debug', 0),
                            'engine': inst['engine'],
                            'ins': [], 'outs': [],
                            'name': f"{inst['name']}-sw{k}",
                            'opcode': 'EventSemaphore',
                            'sync_info': {'on_wait': [extra], 'on_update': []},
                        })
                    si['on_wait'] = [w[-1]]
                out.append(inst)
            bb['instructions'] = out
    return json.dumps(d).encode()


def _install_birpatch():
    import concourse.bass_utils as bu
    import concourse.bass2jax as b2j
    if getattr(bu.compile_bir_kernel, '_waitsplit', False):
        return
    orig = bu.compile_bir_kernel

    def patched(bir_json, tmpdir, neff_name="file.neff"):
        return orig(_split_waits_json(bir_json), tmpdir, neff_name)

    patched._waitsplit = True
    bu.compile_bir_kernel = patched
    b2j.compile_bir_kernel = patched


def _build_bass(reps=1):
    import concourse.bass as bass
    import concourse.tile as tile
    from concourse import mybir

    bf16 = mybir.dt.bfloat16
    f32 = mybir.dt.float32
    AF = mybir.ActivationFunctionType
    ALU = mybir.AluOpType
    AX = mybir.AxisListType

    nc = bass.Bass(trn_type="TRN2")
    xt_d = nc.declare_dram_parameter("xt", [1024, ROWS], bf16, isOutput=False)
    wqkv_d = nc.declare_dram_parameter("wqkv", [1024, 3072], bf16, isOutput=False)
    bias_d = nc.declare_dram_parameter("bqkv", [128, 3072], bf16, isOutput=False)
    wo_d = nc.declare_dram_parameter("wo", [1024, 1024], bf16, isOutput=False)
    bo_d = nc.declare_dram_parameter("bo", [1024, 1], f32, isOutput=False)
    out_d = nc.declare_dram_parameter("out", [1024, ROWS], f32, isOutput=True)

    from contextlib import ExitStack
    with ExitStack() as ctx:
        tc = ctx.enter_context(tile.TileContext(nc))
        const = ctx.enter_context(tc.tile_pool(name="const", bufs=1))
        work = ctx.enter_context(tc.tile_pool(name="work", bufs=2))
        psq = ctx.enter_context(tc.tile_pool(name="psq", bufs=2, space="PSUM"))
        pss = ctx.enter_context(tc.tile_pool(name="pss", bufs=1, space="PSUM"))
        drp = ctx.enter_context(tc.tile_pool(name="drp", bufs=2, space="DRAM"))

        if reps != 1:
            ctx.enter_context(tc.For_i(0, reps))

        # ---- persistent tensors ----
        wq_sb = const.tile([128, 8 * 3072], bf16, tag="wq")
        wo_sb = const.tile([128, 8 * 1024], bf16, tag="wo")
        bias_sb = const.tile([128, 3072], bf16, tag="bias")
        bo_sb = const.tile([128, 8], f32, tag="bo")
        avb2 = const.tile([128, TILES * 1024], bf16, tag="avb2")

        # weight loads: wqkv chunk ec -> cols [3072*ec : +3072]
        src = xt_d  # placeholder to appease linters
        nc.sync.dma_start(
            wq_sb[:].rearrange("p (c f) -> p c f", c=8),
            wqkv_d[:].rearrange("(c p) f -> p c f", c=8),
        )
        nc.sync.dma_start(
            wo_sb[:].rearrange("p (q f) -> p q f", q=8),
            wo_d[:].rearrange("(q p) f -> p q f", q=8),
        )
        nc.sync.dma_start(bias_sb[:], bias_d[:])
        nc.sync.dma_start(
            bo_sb[:],
            bo_d[:].rearrange("(c p) one -> p c one", c=8)[:, :, 0],
        )

        for t in range(TILES):
            # ---- load xT tile: xt_sb[:, 128c:+128] = xt_d[128c:+128, 128t:+128]
            xt_sb = work.tile([128, 1024], bf16, tag="xt")
            nc.sync.dma_start(
                xt_sb[:].rearrange("p (c s) -> p c s", c=8),
                xt_d[:].rearrange("(c p) s -> p c s", c=8)[:, :, 128 * t:128 * (t + 1)],
            )
            # ---- projections (Q, K, V sequentially through 2-bank psum) ----
            qn = work.tile([128, 1024], bf16, tag="qn")
            kn = work.tile([128, 1024], bf16, tag="kn")
            vn = work.tile([128, 1024], bf16, tag="vn")
            for w, dst in enumerate((qn, kn, vn)):
                psp = psq.tile([128, 1024], f32, tag="psp", name="psp")
                for ec in range(8):
                    lhsT = xt_sb[:, 128 * ec:128 * (ec + 1)]
                    for half in range(2):
                        rhs = wq_sb[:, 3072 * ec + 1024 * w + 512 * half:
                                    3072 * ec + 1024 * w + 512 * (half + 1)]
                        nc.tensor.matmul(
                            psp[:, 512 * half:512 * (half + 1)], lhsT, rhs,
                            start=(ec == 0), stop=(ec == 7))
                nc.vector.tensor_add(dst[:], psp[:], bias_sb[:, 1024 * w:1024 * (w + 1)])
            # ---- xbar transposes Q,K ----
            qt = work.tile([128, 1024], bf16, tag="qt")
            kt = work.tile([128, 1024], bf16, tag="kt")
            for c in range(8):
                nc.sync.dma_start_transpose(qt[:, 128 * c:128 * (c + 1)], qn[:, 128 * c:128 * (c + 1)])
                nc.sync.dma_start_transpose(kt[:, 128 * c:128 * (c + 1)], kn[:, 128 * c:128 * (c + 1)])
            # ---- QBLK / KBLK fused copies (double-buffered; re-zero masked) ----
            qblk = work.tile([128, 2048], bf16, tag="qblk", name="qblk")
            kblk = work.tile([128, 1024], bf16, tag="kblk", name="kblk")
            nc.vector.memset(qblk[:], 0.0)
            for p in range(2):
                for i2 in range(2):
                    srcq = qt[:][64 * i2:64 * i2 + 64, :].rearrange(
                        "p (c s) -> p c s", c=8)[:, :, 64 * p:64 * p + 64]
                    dstq = qblk[:][64 * p:64 * p + 64, 1024 * p:1024 * p + 1024].rearrange(
                        "p (c a g) -> p c a g", c=8, a=2)[:, :, i2, :]
                    nc.vector.tensor_copy(dstq, srcq)
                    srck = kt[:][64 * i2:64 * i2 + 64, :].rearrange(
                        "p (c s) -> p c s", c=8)[:, :, 64 * p:64 * p + 64]
                    dstk = kblk[:][64 * p:64 * p + 64, :].rearrange(
                        "p (c a g) -> p c a g", c=8, a=2)[:, :, i2, :]
                    nc.vector.tensor_copy(dstk, srck)
            # ---- scores: 64 pair matmuls ----
            sco = pss.tile([128, 256], f32, tag="sco")
            qv = qblk[:].rearrange("p (a i g) -> p a i g", a=2, i=16)
            kv = kblk[:].rearrange("p (j g) -> p j g", j=16)
            scov = sco[:].rearrange("p (j gg) -> p j gg", j=16)
            for g in range(64):
                u, gg = g % 4, g // 4
                nc.tensor.matmul(
                    scov[32 * u:32 * u + 32, :, gg],
                    qv[:, :, :, g], kv[:, :, g],
                    start=True, stop=True, tile_position=(0, 32 * u))
            # ---- softmax pieces ----
            ex = work.tile([128, 256], bf16, tag="ex")
            nc.scalar.activation(ex[:], sco[:], func=AF.Exp)
            z = work.tile([128, 16], f32, tag="z")
            zr = work.tile([128, 16], f32, tag="zr")
            nc.vector.tensor_reduce(
                z[:], ex[:].rearrange("p (j gg) -> p gg j", j=16),
                axis=AX.X, op=ALU.add)
            nc.vector.reciprocal(zr[:], z[:])
            # ---- bounce E -> ABLK ----
            ablk = work.tile([32, 2048], bf16, tag="ablk", name="ablk")
            vthp = work.tile([32, 4096], bf16, tag="vthp", name="vthp")
            nc.vector.memset(ablk[:], 0.0)
            exd = drp.tile([128, 256], bf16, tag="exd")
            nc.sync.dma_start(exd[:], ex[:])
            for p in range(2):
                for u in range(4):
                    dst = ablk[:][16 * p:16 * p + 16, 1024 * p:1024 * p + 1024].rearrange(
                        "P (i u gg) -> P i u gg", i=16, u=4)[:, :, u, :]
                    srce = exd[:].rearrange(
                        "(u a i) (j gg) -> u a j i gg", u=4, a=2, i=16, j=16)[u, p]
                    nc.sync.dma_start(dst, srce)
            # ---- bounce VN -> VTHP ----
            vnd = drp.tile([128, 1024], bf16, tag="vnd")
            nc.sync.dma_start(vnd[:], vn[:])
            for p in range(2):
                nc.sync.dma_start(
                    vthp[:][16 * p:16 * p + 16, :].rearrange("P (g d) -> P g d", g=64),
                    vnd[:].rearrange("(a g) (j d) -> a j g d", a=2, j=16)[p])
            # ---- AV: 64 pair matmuls ----
            anat = pss.tile([128, 1024], f32, tag="anat")
            av = ablk[:].rearrange("P (a i u gg) -> P a i u gg", a=2, i=16, u=4)
            vv = vthp[:].rearrange("P (g d) -> P g d", g=64)
            anv = anat[:].rearrange("p (gg d) -> p gg d", gg=16)
            for g in range(64):
                u, gg = g % 4, g // 4
                nc.tensor.matmul(
                    anv[32 * u:32 * u + 32, gg, :],
                    av[:, :, :, u, gg], vv[:, g, :],
                    start=True, stop=True, tile_position=(0, 32 * u))
            # ---- normalize by 1/Z and evac ----
            anat_sb = work.tile([128, 1024], bf16, tag="anat_sb")
            asv = anat_sb[:].rearrange("p (gg d) -> p gg d", gg=16)
            for gg in range(16):
                nc.vector.tensor_scalar_mul(asv[:, gg, :], anv[:, gg, :], zr[:, gg:gg + 1])
            # ---- xbar ANAT -> AVB2 ----
            for c4 in range(8):
                nc.sync.dma_start_transpose(
                    avb2[:, 1024 * t + 128 * c4:1024 * t + 128 * (c4 + 1)],
                    anat_sb[:, 128 * c4:128 * (c4 + 1)])

        # ---- final projection ----
        avv = avb2[:].rearrange("p (t c u a i) -> p t c u a i", t=TILES, c=8, u=4, a=2)
        for fc in range(8):
            for tg in range(4):
                psf = pss.tile([128, 512], f32, tag="psf")
                for q in range(8):
                    u, pq = q // 2, q % 2
                    nc.tensor.matmul(
                        psf[:], wo_sb[:, 1024 * q + 128 * fc:1024 * q + 128 * (fc + 1)],
                        avv[:, 4 * tg:4 * (tg + 1), :, u, pq, :],
                        start=(q == 0), stop=(q == 7))
                osb = work.tile([128, 512], f32, tag="osb")
                nc.vector.tensor_scalar_add(osb[:], psf[:], bo_sb[:, fc:fc + 1])
                nc.sync.dma_start(out_d[128 * fc:128 * (fc + 1), 512 * tg:512 * (tg + 1)], osb[:])
    return nc


def _host_prep(x, Wq, bq, Wk, bk, Wv, bv, Wo, bo):
    """Returns per-core input maps + post-scatter metadata."""
    xf = np.ascontiguousarray(x.reshape(NB * SB, EB))
    # slot permutation within each 128-tile: col 128t + 64p + g <- row 128t+2g+p
    idx = np.arange(ROWS)
    t, r = idx // 128, idx % 128
    p, g = r // 64, r % 64
    perm = 128 * t + 2 * g + p
    Wqs, bqs = Wq / 8.0, bq / 8.0
    WQKVT = np.concatenate([Wqs.T, Wk.T, Wv.T], axis=1).astype(ml_dtypes.bfloat16)
    BQKV = np.tile(np.concatenate([bqs, bk, bv])[None, :], (128, 1)).astype(ml_dtypes.bfloat16)
    WoTP = np.zeros((1024, 1024), np.float32)
    for u in range(4):
        for pp in range(2):
            q = 2 * u + pp
            for b in range(2):
                m = 8 * b + 2 * u + pp
                WoTP[q * 128 + b * 64:q * 128 + (b + 1) * 64, :] = Wo[:, m * 64:(m + 1) * 64].T
    WoTP = WoTP.astype(ml_dtypes.bfloat16)
    boT = bo.reshape(1024, 1).astype(np.float32)
    in_maps = []
    for core in range(8):
        n, s0 = core // 2, (core % 2) * ROWS
        xs = xf[n * SB + s0:n * SB + s0 + ROWS][perm]
        xT = np.ascontiguousarray(xs.T).astype(ml_dtypes.bfloat16)
        in_maps.append({"xt": xT, "wqkv": WQKVT, "bqkv": BQKV, "wo": WoTP, "bo": boT})
    return in_maps


def _gather_out(core_outs):
    """core_outs: list of 8 per-core 'out' arrays (1024, 2048) -> full (N,S,E)."""
    out = np.zeros((NB, SB, EB), np.float32)
    # col t*128 + c4*16 + h -> row s_out = h*256 + (s0/16 + 8t + c4)
    tt = np.arange(ROWS)
    ct, cc4, ch = tt // 128, (tt // 16) % 8, tt % 16
    for core in range(8):
        n, s0 = core // 2, (core % 2) * ROWS
        fT = np.asarray(core_outs[core])  # (1024, 2048)
        rows = ch * 256 + (s0 // 16 + 8 * ct + cc4)
        out[n, rows, :] = fT.T
    return out


def kernel(x, Wq, bq, Wk, bk, Wv, bv, Wo, bo):
    _install_birpatch()
    from concourse.bass_utils import run_bass_kernel_spmd

    if "nc" not in _CACHE:
        _CACHE["nc"] = _build_bass()
    nc = _CACHE["nc"]
    in_maps = _host_prep(np.asarray(x, np.float32), *[np.asarray(a, np.float32)
                         for a in (Wq, bq, Wk, bk, Wv, bv, Wo, bo)])
    res = run_bass_kernel_spmd(nc, in_maps, list(range(8)))
    return _gather_out([res.results[core]["out"] for core in range(8)])



# revision 28
# speedup vs baseline: 131.9751x; 1.9758x over previous
"""Trainium2 Bass kernel for nn_MultiHeadSelfAttention_62646392979761.

Math (per the buggy-einsum reference): per position s, heads attend to heads:
  Q,K,V = x@W{q,k,v}.T + b  (N,S,H,D);  scores[s] = Q[s]K[s]^T/8 (16x16);
  A = softmax_j;  AV[s] = A[s]V[s];  out2 = scrambled flat reshape;
  final = out2@Wo.T + bo.

Sharding: 8 cores x 2048 rows of the flattened (16384, 1024) x. Attention is
position-local; the scramble groups 16 consecutive positions, which never
cross a 2048-row shard. Zero cross-core communication.

Per-core pipeline (positions in 4 groups of 512, each 4 subtiles of 128).
Indices: position s = 128*sub + 16*G + w, pair-half p = w%2, c' = w//2%8,
pair g = 8*G + c' (u = g%4, gg = g//4), head i = 2c+a.
  1. QT/KT projections transposed (stationary = weight chunk, moving = xT):
     psum [128 f=(a,d), 512 s] per f-chunk; DVE bias evac -> qt/kt [(a,d),(c,s)].
  2. V projection natural (stationary = xT chunk): vn [128 s, 1024 (j,d)].
  3. QSTK [(p,d), 16g+i] / KBLK [(p,d), 32g+16p+j block-diag] via 4 strided
     DVE copies each from qt/kt. VBLK [(p,j), 128g+64p+d block-diag] via 2
     SBUF->SBUF gather DMAs from vn. Structural zeros memset on first use.
  4. Scores^T: 64 pair matmuls (k=128, m=32 col-rotated, n=16, stationary
     KBLK slab, moving QSTK slice) -> sco psum [(u,p,j), (gg,i)].
  5. Softmax: ACT exp -> E bf16; Z = mask^T-matmul (sums j over partitions);
     DVE reciprocal; Zb = maskT-matmul broadcast; A = E*Zb (DVE).
  6. AV: 64 pair matmuls (k=32 row-rotated, m=128, n=16, stationary VBLK
     slab, moving A slice) -> avp psum [(p,d), 16g+i].
  7. ACT copies avp -> out2T chunks [128 (a,d), 2048 (16G+i)] (c' = chunk).
  8. Final projection: stationary WoPT chunks, moving out2T -> psum
     [128 f, 512 s'], + bo (DVE) -> out_d (1024, 2048) f32.
Host: transposes x/weights, permutes Wo rows, post-scatters out columns
(col = 16G + i -> row i*256 + s0/16 + G).
"""

import math
import numpy as np
import ml_dtypes

ROWS = 2048
NB, SB, EB, HB, DB = 4, 4096, 1024, 16, 64

_CACHE = {}


def _split_waits_json(bir_bytes):
    """This env's walrus accepts only ONE embedded sync-wait per TPB
    instruction (NEURON_ISA_TPB_EVENTS has a single wait slot) but Tile emits
    several. Split excess on_wait entries onto standalone EventSemaphore
    instructions inserted just before, on the same engine — semantically
    identical on in-order engine queues."""
    import json
    d = json.loads(bir_bytes)
    for fn in d.get('functions', []):
        for bb in (fn.get('basic_blocks') or fn.get('blocks') or []):
            out = []
            for inst in bb.get('instructions', []):
                si = inst.get('sync_info')
                w = (si or {}).get('on_wait') or []
                if len(w) > 1:
                    for k, extra in enumerate(w[:-1]):
                        out.append({
                            'debug': inst.get('debug', 0),
                            'engine': inst['engine'],
                            'ins': [], 'outs': [],
                            'name': f"{inst['name']}-sw{k}",
                            'opcode': 'EventSemaphore',
                            'sync_info': {'on_wait': [extra], 'on_update': []},
                        })
                    si['on_wait'] = [w[-1]]
                out.append(inst)
            bb['instructions'] = out
    return json.dumps(d).encode()


def _install_birpatch():
    import concourse.bass_utils as bu
    import concourse.bass2jax as b2j
    if getattr(bu.compile_bir_kernel, '_waitsplit', False):
        return
    orig = bu.compile_bir_kernel

    def patched(bir_json, tmpdir, neff_name="file.neff"):
        return orig(_split_waits_json(bir_json), tmpdir, neff_name)

    patched._waitsplit = True
    bu.compile_bir_kernel = patched
    b2j.compile_bir_kernel = patched


def _build_bass(reps=1):
    import os
    import concourse.bass as bass
    import concourse.tile as tile
    from concourse import mybir
    _ab = set(os.environ.get("KABLATE", "").split(","))

    bf16 = mybir.dt.bfloat16
    f32 = mybir.dt.float32
    AF = mybir.ActivationFunctionType

    nc = bass.Bass(trn_type="TRN2")
    xt_d = nc.declare_dram_parameter("xt", [1024, ROWS], bf16, isOutput=False)
    wqT_d = nc.declare_dram_parameter("wqt", [1024, 1024], bf16, isOutput=False)
    wkT_d = nc.declare_dram_parameter("wkt", [1024, 1024], bf16, isOutput=False)
    wvT_d = nc.declare_dram_parameter("wvt", [1024, 1024], bf16, isOutput=False)
    woT_d = nc.declare_dram_parameter("wot", [1024, 1024], bf16, isOutput=False)
    bias_d = nc.declare_dram_parameter("bias", [128, 1048], f32, isOutput=False)
    mask_d = nc.declare_dram_parameter("mask", [128, 32], bf16, isOutput=False)
    maskT_d = nc.declare_dram_parameter("maskt", [32, 128], bf16, isOutput=False)
    out_d = nc.declare_dram_parameter("out", [1024, ROWS], f32, isOutput=True)

    from contextlib import ExitStack
    with ExitStack() as ctx:
        tc = ctx.enter_context(tile.TileContext(nc))
        const = ctx.enter_context(tc.tile_pool(name="const", bufs=1))
        work = ctx.enter_context(tc.tile_pool(name="work", bufs=2))
        vnp = ctx.enter_context(tc.tile_pool(name="vnp", bufs=2))
        ppj = ctx.enter_context(tc.tile_pool(name="ppj", bufs=1, space="PSUM"))
        psc = ctx.enter_context(tc.tile_pool(name="psc", bufs=2, space="PSUM"))
        ps1 = ctx.enter_context(tc.tile_pool(name="ps1", bufs=1, space="PSUM"))
        drp = ctx.enter_context(tc.tile_pool(name="drp", bufs=4, space="DRAM"))

        if reps != 1:
            ctx.enter_context(tc.For_i(0, reps))

        # ---- persistent tensors (reloaded every rep for honest timing) ----
        wq_sb = const.tile([128, 8192], bf16, tag="wq")
        wk_sb = const.tile([128, 8192], bf16, tag="wk")
        wv_sb = const.tile([128, 8192], bf16, tag="wv")
        wo_sb = const.tile([128, 8192], bf16, tag="wo")
        bias_sb = const.tile([128, 1048], f32, tag="bias")
        mask_sb = const.tile([128, 32], bf16, tag="mask")
        maskT_sb = const.tile([32, 128], bf16, tag="maskt")
        out2t = [const.tile([128, 2048], bf16, tag=f"o2t{c}", name=f"o2t{c}")
                 for c in range(8)]
        kblks = [const.tile([128, 2048], bf16, tag=f"kblk{b}", name=f"kblk{b}")
                 for b in range(2)]
        vblks = [const.tile([32, 8192], bf16, tag=f"vblk{b}", name=f"vblk{b}")
                 for b in range(2)]
        for b in range(2):
            nc.gpsimd.memset(kblks[b][:], 0.0)
            nc.gpsimd.memset(vblks[b][:], 0.0)

        # weight layout: sb[p, 1024*ce + 128*cf + m] = W_T[128*ce + p, 128*cf + m]
        for sb, d in ((wq_sb, wqT_d), (wk_sb, wkT_d), (wv_sb, wvT_d), (wo_sb, woT_d)):
            nc.sync.dma_start(
                sb[:].rearrange("p (ce f) -> p ce f", ce=8),
                d[:].rearrange("(ce p) f -> p ce f", ce=8),
            )
        nc.sync.dma_start(bias_sb[:], bias_d[:])
        nc.sync.dma_start(mask_sb[:], mask_d[:])
        nc.sync.dma_start(maskT_sb[:], maskT_d[:])
        bqT = bias_sb[:, 0:8]
        bkT = bias_sb[:, 8:16]
        boT = bias_sb[:, 16:24]
        bvR = bias_sb[:, 24:1048]

        for sg in range(4):
            # ---- load xT group: [128, (ce, s 512)] ----
            xt_sb = work.tile([128, 4096], bf16, tag="xt")
            nc.sync.dma_start(
                xt_sb[:].rearrange("p (ce s) -> p ce s", ce=8),
                xt_d[:].rearrange("(ce p) s -> p ce s", ce=8)[:, :, 512 * sg:512 * (sg + 1)],
            )
            # ---- QT / KT projections (transposed: stationary = weights) ----
            qt_sg = work.tile([128, 4096], bf16, tag="qt")
            kt_sg = work.tile([128, 4096], bf16, tag="kt")
            for w_sb, bT, dst in ((wq_sb, bqT, qt_sg), (wk_sb, bkT, kt_sg)):
                for cf in range(8):
                    pp = ppj.tile([128, 512], f32, tag="pp", name="pp")
                    for ce in range(8):
                        nc.tensor.matmul(
                            pp[:], w_sb[:, 1024 * ce + 128 * cf:1024 * ce + 128 * (cf + 1)],
                            xt_sb[:, 512 * ce:512 * (ce + 1)],
                            start=(ce == 0), stop=(ce == 7))
                    nc.vector.tensor_scalar_add(
                        dst[:, 512 * cf:512 * (cf + 1)], pp[:], bT[:, cf:cf + 1])
            # ---- V projection (natural: stationary = xT chunk) ----
            vns = []
            for sl in range(4):
                vn = vnp.tile([128, 1024], bf16, tag="vn")
                for h in range(2):
                    pv = ppj.tile([128, 512], f32, tag="pv", name="pv")
                    for ce in range(8):
                        nc.tensor.matmul(
                            pv[:],
                            xt_sb[:, 512 * ce + 128 * sl:512 * ce + 128 * (sl + 1)],
                            wv_sb[:, 1024 * ce + 512 * h:1024 * ce + 512 * (h + 1)],
                            start=(ce == 0), stop=(ce == 7))
                    nc.vector.tensor_add(vn[:, 512 * h:512 * (h + 1)], pv[:],
                                         bvR[:, 512 * h:512 * (h + 1)])
                vdr = drp.tile([128, 1024], bf16, tag="vdr")
                nc.sync.dma_start(vdr[:], vn[:])
                vns.append(vdr)

            for sl in range(4):
                sub = 4 * sg + sl
                vdr = vns[sl]
                # ---- VBLK [32 (p,j), 128g+64p+d] via 2 gather DMAs (DRAM src) ----
                vblk = vblks[sub % 2]
                for p in range(2 if "novblk" not in _ab else 0):
                    nc.sync.dma_start(
                        vblk[:][16 * p:16 * (p + 1), :].rearrange(
                            "P (g two d) -> P g two d", g=64, two=2)[:, :, p, :],
                        vdr[:].rearrange("(G cp t) (j d) -> t j G cp d",
                                         G=8, cp=8, t=2, j=16)[p],
                    )
                # ---- QSTK / KBLK via 4 strided DVE copies each ----
                qstk = work.tile([128, 1024], bf16, tag="qstk", name="qstk")
                kblk = kblks[sub % 2]
                for p in range(2):
                    for a in range(2):
                        src_q = qt_sg[:][64 * a:64 * (a + 1), :].rearrange(
                            "P (c sl G cp t) -> P sl t G cp c",
                            c=8, sl=4, G=8, cp=8, t=2)[:, sl, p]
                        dst_q = qstk[:][64 * p:64 * (p + 1), :].rearrange(
                            "P (G cp c t) -> P t G cp c", G=8, cp=8, c=8, t=2)[:, a]
                        nc.vector.tensor_copy(dst_q, src_q)
                        src_k = kt_sg[:][64 * a:64 * (a + 1), :].rearrange(
                            "P (c sl G cp t) -> P sl t G cp c",
                            c=8, sl=4, G=8, cp=8, t=2)[:, sl, p]
                        dst_k = kblk[:][64 * p:64 * (p + 1), :].rearrange(
                            "P (G cp two c t) -> P two t G cp c",
                            G=8, cp=8, two=2, c=8, t=2)[:, p, a]
                        nc.vector.tensor_copy(dst_k, src_k)
                # ---- scores^T: 64 pair matmuls ----
                sco = psc.tile([128, 256], f32, tag="sco", name="sco")
                if "nosco" in _ab:
                    nc.scalar.activation(sco[:], qstk[:, 0:256], func=AF.Copy)
                else:
                    for g in range(64):
                        u, gg = g % 4, g // 4
                        nc.tensor.matmul(
                            sco[32 * u:32 * (u + 1), 16 * gg:16 * (gg + 1)],
                            kblk[:, 32 * g:32 * (g + 1)],
                            qstk[:, 16 * g:16 * (g + 1)],
                            start=True, stop=True, tile_position=(0, 32 * u))
                # ---- softmax ----
                e_sb = work.tile([128, 256], bf16, tag="esb")
                nc.scalar.activation(e_sb[:], sco[:], func=AF.Exp)
                a_sb = work.tile([128, 256], bf16, tag="asb")
                if "noz" in _ab:
                    nc.vector.tensor_copy(a_sb[:], e_sb[:])
                else:
                    zp = ps1.tile([128, 512], f32, tag="zp", name="zp")
                    nc.tensor.matmul(zp[0:32, 0:256], mask_sb[:], e_sb[:],
                                     start=True, stop=True)
                    zr = work.tile([32, 256], bf16, tag="zr")
                    nc.vector.memset(zr[:], 0.0)
                    with nc.allow_low_precision(reason="1/Z in bf16: 0.4% on softmax scale"):
                        nc.vector.reciprocal(zr[0:8, :], zp[0:8, 0:256])
                    nc.tensor.matmul(zp[:, 256:512], maskT_sb[:], zr[:],
                                     start=True, stop=True)
                    nc.vector.tensor_mul(a_sb[:], e_sb[:], zp[:, 256:512])
                # ---- A2: bands stacked at partition base 0 ----
                a2 = work.tile([32, 1024], bf16, tag="a2")
                for u in range(4):
                    nc.vector.tensor_copy(a2[:, 256 * u:256 * (u + 1)],
                                          a_sb[32 * u:32 * (u + 1), :])
                # ---- AV: 64 pair matmuls ----
                avp = ps1.tile([128, 1024], f32, tag="avp", name="avp")
                if "noav" in _ab:
                    for q in range(4):
                        nc.scalar.activation(avp[:, 256 * q:256 * (q + 1)], a_sb[:],
                                             func=AF.Copy)
                else:
                    for g in range(64):
                        u, gg = g % 4, g // 4
                        nc.tensor.matmul(
                            avp[:, 16 * g:16 * (g + 1)],
                            vblk[:, 128 * g:128 * (g + 1)],
                            a2[:, 256 * u + 16 * gg:256 * u + 16 * (gg + 1)],
                            start=True, stop=True)
                # ---- evac to out2T chunks ----
                for cp in range(8):
                    nc.scalar.activation(
                        out2t[cp][:, 128 * sub:128 * (sub + 1)],
                        avp[:].rearrange("P (G cp i) -> P cp G i", G=8, cp=8)[:, cp],
                        func=AF.Copy)

            # ---- final projection for this group's s' columns ----
            for cf in range(8):
                pf = ps1.tile([128, 512], f32, tag="pf", name="pf")
                for cp in range(8):
                    nc.tensor.matmul(
                        pf[:], wo_sb[:, 1024 * cp + 128 * cf:1024 * cp + 128 * (cf + 1)],
                        out2t[cp][:, 512 * sg:512 * (sg + 1)],
                        start=(cp == 0), stop=(cp == 7))
                osb = work.tile([128, 512], f32, tag="osb")
                nc.vector.tensor_scalar_add(osb[:], pf[:], boT[:, cf:cf + 1])
                nc.sync.dma_start(out_d[128 * cf:128 * (cf + 1), 512 * sg:512 * (sg + 1)], osb[:])
    return nc


def _host_prep(x, Wq, bq, Wk, bk, Wv, bv, Wo, bo):
    """Returns per-core input maps."""
    xf = np.ascontiguousarray(x.reshape(NB * SB, EB))
    WqT = np.ascontiguousarray((Wq / 8.0).T).astype(ml_dtypes.bfloat16)
    WkT = np.ascontiguousarray(Wk.T).astype(ml_dtypes.bfloat16)
    WvT = np.ascontiguousarray(Wv.T).astype(ml_dtypes.bfloat16)
    WoPT = np.zeros((1024, 1024), np.float32)
    for cp in range(8):
        for a in range(2):
            w = 2 * cp + a
            WoPT[128 * cp + 64 * a:128 * cp + 64 * a + 64, :] = Wo[:, 64 * w:64 * (w + 1)].T
    WoPT = WoPT.astype(ml_dtypes.bfloat16)
    bias = np.zeros((128, 1048), np.float32)
    bias[:, 0:8] = (bq / 8.0).reshape(8, 128).T
    bias[:, 8:16] = bk.reshape(8, 128).T
    bias[:, 16:24] = bo.reshape(8, 128).T
    bias[:, 24:1048] = np.tile(bv[None, :], (128, 1))
    MASK = np.zeros((128, 32), np.float32)
    for u in range(4):
        for p in range(2):
            MASK[32 * u + 16 * p:32 * u + 16 * (p + 1), 2 * u + p] = 1.0
    MASKb = MASK.astype(ml_dtypes.bfloat16)
    MASKTb = np.ascontiguousarray(MASK.T).astype(ml_dtypes.bfloat16)
    in_maps = []
    for core in range(8):
        n, s0 = core // 2, (core % 2) * ROWS
        xs = xf[n * SB + s0:n * SB + s0 + ROWS]
        xT = np.ascontiguousarray(xs.T).astype(ml_dtypes.bfloat16)
        in_maps.append({"xt": xT, "wqt": WqT, "wkt": WkT, "wvt": WvT,
                        "wot": WoPT, "bias": bias, "mask": MASKb, "maskt": MASKTb})
    return in_maps


def _gather_out(core_outs):
    """core_outs: list of 8 per-core 'out' arrays (1024, 2048) -> full (N,S,E).
    out col = 16*G + i -> row i*256 + s0/16 + G."""
    out = np.zeros((NB, SB, EB), np.float32)
    cols = np.arange(ROWS)
    G, i = cols // 16, cols % 16
    for core in range(8):
        n, s0 = core // 2, (core % 2) * ROWS
        fT = np.asarray(core_outs[core])  # (1024, 2048)
        rows = i * 256 + (s0 // 16 + G)
        out[n, rows, :] = fT.T
    return out


def kernel(x, Wq, bq, Wk, bk, Wv, bv, Wo, bo):
    _install_birpatch()
    from concourse.bass_utils import run_bass_kernel_spmd

    if "nc" not in _CACHE:
        _CACHE["nc"] = _build_bass()
    nc = _CACHE["nc"]
    in_maps = _host_prep(np.asarray(x, np.float32), *[np.asarray(a, np.float32)
                         for a in (Wq, bq, Wk, bk, Wv, bv, Wo, bo)])
    res = run_bass_kernel_spmd(nc, in_maps, list(range(8)))
    return _gather_out([res.results[core]["out"] for core in range(8)])


# revision 30
# speedup vs baseline: 133.5039x; 1.0116x over previous
"""Trainium2 Bass kernel for nn_MultiHeadSelfAttention_62646392979761.

Math (per the buggy-einsum reference): per position s, heads attend to heads:
  Q,K,V = x@W{q,k,v}.T + b  (N,S,H,D);  scores[s] = Q[s]K[s]^T/8 (16x16);
  A = softmax_j;  AV[s] = A[s]V[s];  out2 = scrambled flat reshape;
  final = out2@Wo.T + bo.

Sharding: 8 cores x 2048 rows of the flattened (16384, 1024) x. Attention is
position-local; the scramble groups 16 consecutive positions, which never
cross a 2048-row shard. Zero cross-core communication.

Per-core pipeline (positions in 4 groups of 512, each 4 subtiles of 128).
Indices: position s = 128*sub + 16*G + w, pair-half p = w%2, c' = w//2%8,
pair g = 8*G + c' (u = g%4, gg = g//4), head i = 2c+a.
  1. QT/KT projections transposed (stationary = weight chunk, moving = xT):
     psum [128 f=(a,d), 512 s] per f-chunk; DVE bias evac -> qt/kt [(a,d),(c,s)].
  2. V projection natural (stationary = xT chunk): vn [128 s, 1024 (j,d)].
  3. QSTK [(p,d), 16g+i] / KBLK [(p,d), 32g+16p+j block-diag] via 4 strided
     DVE copies each from qt/kt. VBLK [(p,j), 128g+64p+d block-diag] via 2
     SBUF->SBUF gather DMAs from vn. Structural zeros memset on first use.
  4. Scores^T: 64 pair matmuls (k=128, m=32 col-rotated, n=16, stationary
     KBLK slab, moving QSTK slice) -> sco psum [(u,p,j), (gg,i)].
  5. Softmax: ACT exp -> E bf16; Z = mask^T-matmul (sums j over partitions);
     DVE reciprocal; Zb = maskT-matmul broadcast; A = E*Zb (DVE).
  6. AV: 64 pair matmuls (k=32 row-rotated, m=128, n=16, stationary VBLK
     slab, moving A slice) -> avp psum [(p,d), 16g+i].
  7. ACT copies avp -> out2T chunks [128 (a,d), 2048 (16G+i)] (c' = chunk).
  8. Final projection: stationary WoPT chunks, moving out2T -> psum
     [128 f, 512 s'], + bo (DVE) -> out_d (1024, 2048) f32.
Host: transposes x/weights, permutes Wo rows, post-scatters out columns
(col = 16G + i -> row i*256 + s0/16 + G).
"""

import math
import numpy as np
import ml_dtypes

ROWS = 2048
NB, SB, EB, HB, DB = 4, 4096, 1024, 16, 64

_CACHE = {}


def _split_waits_json(bir_bytes):
    """This env's walrus accepts only ONE embedded sync-wait per TPB
    instruction (NEURON_ISA_TPB_EVENTS has a single wait slot) but Tile emits
    several. Split excess on_wait entries onto standalone EventSemaphore
    instructions inserted just before, on the same engine — semantically
    identical on in-order engine queues."""
    import json
    d = json.loads(bir_bytes)
    for fn in d.get('functions', []):
        for bb in (fn.get('basic_blocks') or fn.get('blocks') or []):
            out = []
            for inst in bb.get('instructions', []):
                si = inst.get('sync_info')
                w = (si or {}).get('on_wait') or []
                if len(w) > 1:
                    for k, extra in enumerate(w[:-1]):
                        out.append({
                            'debug': inst.get('debug', 0),
                            'engine': inst['engine'],
                            'ins': [], 'outs': [],
                            'name': f"{inst['name']}-sw{k}",
                            'opcode': 'EventSemaphore',
                            'sync_info': {'on_wait': [extra], 'on_update': []},
                        })
                    si['on_wait'] = [w[-1]]
                out.append(inst)
            bb['instructions'] = out
    return json.dumps(d).encode()


def _install_birpatch():
    import concourse.bass_utils as bu
    import concourse.bass2jax as b2j
    if getattr(bu.compile_bir_kernel, '_waitsplit', False):
        return
    orig = bu.compile_bir_kernel

    def patched(bir_json, tmpdir, neff_name="file.neff"):
        return orig(_split_waits_json(bir_json), tmpdir, neff_name)

    patched._waitsplit = True
    bu.compile_bir_kernel = patched
    b2j.compile_bir_kernel = patched


def _build_bass(reps=1):
    import os
    import concourse.bass as bass
    import concourse.tile as tile
    from concourse import mybir
    _ab = set(os.environ.get("KABLATE", "").split(","))

    bf16 = mybir.dt.bfloat16
    f32 = mybir.dt.float32
    AF = mybir.ActivationFunctionType

    nc = bass.Bass(trn_type="TRN2")
    xt_d = nc.declare_dram_parameter("xt", [1024, ROWS], bf16, isOutput=False)
    wqT_d = nc.declare_dram_parameter("wqt", [1024, 1024], bf16, isOutput=False)
    wkT_d = nc.declare_dram_parameter("wkt", [1024, 1024], bf16, isOutput=False)
    wvT_d = nc.declare_dram_parameter("wvt", [1024, 1024], bf16, isOutput=False)
    woT_d = nc.declare_dram_parameter("wot", [1024, 1024], bf16, isOutput=False)
    bias_d = nc.declare_dram_parameter("bias", [128, 1048], f32, isOutput=False)
    mask_d = nc.declare_dram_parameter("mask", [128, 32], bf16, isOutput=False)
    maskT_d = nc.declare_dram_parameter("maskt", [32, 128], bf16, isOutput=False)
    out_d = nc.declare_dram_parameter("out", [1024, ROWS], f32, isOutput=True)

    from contextlib import ExitStack
    with ExitStack() as ctx:
        tc = ctx.enter_context(tile.TileContext(nc))
        const = ctx.enter_context(tc.tile_pool(name="const", bufs=1))
        work = ctx.enter_context(tc.tile_pool(name="work", bufs=2))
        vnp = ctx.enter_context(tc.tile_pool(name="vnp", bufs=2))
        ppj = ctx.enter_context(tc.tile_pool(name="ppj", bufs=1, space="PSUM"))
        psc = ctx.enter_context(tc.tile_pool(name="psc", bufs=1, space="PSUM"))
        ps2 = ctx.enter_context(tc.tile_pool(name="ps2", bufs=2, space="PSUM"))
        ps1 = ctx.enter_context(tc.tile_pool(name="ps1", bufs=1, space="PSUM"))
        drp = ctx.enter_context(tc.tile_pool(name="drp", bufs=4, space="DRAM"))

        if reps != 1:
            ctx.enter_context(tc.For_i(0, reps))

        # ---- persistent tensors (reloaded every rep for honest timing) ----
        wq_sb = const.tile([128, 8192], bf16, tag="wq")
        wk_sb = const.tile([128, 8192], bf16, tag="wk")
        wv_sb = const.tile([128, 8192], bf16, tag="wv")
        wo_sb = const.tile([128, 8192], bf16, tag="wo")
        bias_sb = const.tile([128, 1048], f32, tag="bias")
        mask_sb = const.tile([128, 32], bf16, tag="mask")
        maskT_sb = const.tile([32, 128], bf16, tag="maskt")
        out2t = [const.tile([128, 2048], bf16, tag=f"o2t{c}", name=f"o2t{c}")
                 for c in range(8)]
        kblks = [const.tile([128, 2048], bf16, tag=f"kblk{b}", name=f"kblk{b}")
                 for b in range(2)]
        vblks = [const.tile([32, 8192], bf16, tag=f"vblk{b}", name=f"vblk{b}")
                 for b in range(2)]
        for b in range(2):
            nc.gpsimd.memset(kblks[b][:], 0.0)
            nc.gpsimd.memset(vblks[b][:], 0.0)

        # weight layout: sb[p, 1024*ce + 128*cf + m] = W_T[128*ce + p, 128*cf + m]
        for sb, d in ((wq_sb, wqT_d), (wk_sb, wkT_d), (wv_sb, wvT_d), (wo_sb, woT_d)):
            nc.sync.dma_start(
                sb[:].rearrange("p (ce f) -> p ce f", ce=8),
                d[:].rearrange("(ce p) f -> p ce f", ce=8),
            )
        nc.sync.dma_start(bias_sb[:], bias_d[:])
        nc.sync.dma_start(mask_sb[:], mask_d[:])
        nc.sync.dma_start(maskT_sb[:], maskT_d[:])
        bqT = bias_sb[:, 0:8]
        bkT = bias_sb[:, 8:16]
        boT = bias_sb[:, 16:24]
        bvR = bias_sb[:, 24:1048]

        def _emit_final(fg):
            for cf in range(8):
                pf = ps1.tile([128, 512], f32, tag="zpf", name="pf")
                for cp in range(8):
                    nc.tensor.matmul(
                        pf[:], wo_sb[:, 1024 * cp + 128 * cf:1024 * cp + 128 * (cf + 1)],
                        out2t[cp][:, 512 * fg:512 * (fg + 1)],
                        start=(cp == 0), stop=(cp == 7))
                osb = work.tile([128, 512], f32, tag="osb")
                nc.vector.tensor_scalar_add(osb[:], pf[:], boT[:, cf:cf + 1])
                nc.sync.dma_start(out_d[128 * cf:128 * (cf + 1), 512 * fg:512 * (fg + 1)], osb[:])

        for sg in range(4):
            # ---- load xT group: [128, (ce, s 512)] ----
            xt_sb = work.tile([128, 4096], bf16, tag="xt")
            nc.sync.dma_start(
                xt_sb[:].rearrange("p (ce s) -> p ce s", ce=8),
                xt_d[:].rearrange("(ce p) s -> p ce s", ce=8)[:, :, 512 * sg:512 * (sg + 1)],
            )
            # ---- QT / KT projections (transposed: stationary = weights) ----
            qt_sg = work.tile([128, 4096], bf16, tag="qt")
            kt_sg = work.tile([128, 4096], bf16, tag="kt")
            for w_sb, bT, dst in ((wq_sb, bqT, qt_sg), (wk_sb, bkT, kt_sg)):
                for cf in range(8):
                    pp = ppj.tile([128, 512], f32, tag="pp", name="pp")
                    for ce in range(8):
                        nc.tensor.matmul(
                            pp[:], w_sb[:, 1024 * ce + 128 * cf:1024 * ce + 128 * (cf + 1)],
                            xt_sb[:, 512 * ce:512 * (ce + 1)],
                            start=(ce == 0), stop=(ce == 7))
                    nc.vector.tensor_scalar_add(
                        dst[:, 512 * cf:512 * (cf + 1)], pp[:], bT[:, cf:cf + 1])
            # ---- V projection (natural: stationary = xT chunk) ----
            vns = []
            for sl in range(4):
                vn = vnp.tile([128, 1024], bf16, tag="vn")
                for h in range(2):
                    pv = ppj.tile([128, 512], f32, tag="pv", name="pv")
                    for ce in range(8):
                        nc.tensor.matmul(
                            pv[:],
                            xt_sb[:, 512 * ce + 128 * sl:512 * ce + 128 * (sl + 1)],
                            wv_sb[:, 1024 * ce + 512 * h:1024 * ce + 512 * (h + 1)],
                            start=(ce == 0), stop=(ce == 7))
                    nc.vector.tensor_add(vn[:, 512 * h:512 * (h + 1)], pv[:],
                                         bvR[:, 512 * h:512 * (h + 1)])
                vdr = drp.tile([128, 1024], bf16, tag="vdr")
                nc.sync.dma_start(vdr[:], vn[:])
                vns.append(vdr)

            for sl in range(4):
                sub = 4 * sg + sl
                vdr = vns[sl]
                # ---- VBLK [32 (p,j), 128g+64p+d] via 2 gather DMAs (DRAM src) ----
                vblk = vblks[sub % 2]
                for p in range(2 if "novblk" not in _ab else 0):
                    nc.sync.dma_start(
                        vblk[:][16 * p:16 * (p + 1), :].rearrange(
                            "P (g two d) -> P g two d", g=64, two=2)[:, :, p, :],
                        vdr[:].rearrange("(G cp t) (j d) -> t j G cp d",
                                         G=8, cp=8, t=2, j=16)[p],
                    )
                # ---- QSTK / KBLK via 4 strided DVE copies each ----
                qstk = work.tile([128, 1024], bf16, tag="qstk", name="qstk")
                kblk = kblks[sub % 2]
                for p in range(2):
                    for a in range(2):
                        src_q = qt_sg[:][64 * a:64 * (a + 1), :].rearrange(
                            "P (c sl G cp t) -> P sl t G cp c",
                            c=8, sl=4, G=8, cp=8, t=2)[:, sl, p]
                        dst_q = qstk[:][64 * p:64 * (p + 1), :].rearrange(
                            "P (G cp c t) -> P t G cp c", G=8, cp=8, c=8, t=2)[:, a]
                        nc.vector.tensor_copy(dst_q, src_q)
                        src_k = kt_sg[:][64 * a:64 * (a + 1), :].rearrange(
                            "P (c sl G cp t) -> P sl t G cp c",
                            c=8, sl=4, G=8, cp=8, t=2)[:, sl, p]
                        dst_k = kblk[:][64 * p:64 * (p + 1), :].rearrange(
                            "P (G cp two c t) -> P two t G cp c",
                            G=8, cp=8, two=2, c=8, t=2)[:, p, a]
                        nc.vector.tensor_copy(dst_k, src_k)
                # ---- scores^T: 64 pair matmuls ----
                sco = psc.tile([128, 256], f32, tag="sco", name="sco")
                if "nosco" in _ab:
                    nc.scalar.activation(sco[:], qstk[:, 0:256], func=AF.Copy)
                else:
                    for g in range(64):
                        u, gg = g % 4, g // 4
                        nc.tensor.matmul(
                            sco[32 * u:32 * (u + 1), 16 * gg:16 * (gg + 1)],
                            kblk[:, 32 * g:32 * (g + 1)],
                            qstk[:, 16 * g:16 * (g + 1)],
                            start=True, stop=True, tile_position=(0, 32 * u))
                # ---- softmax ----
                e_sb = work.tile([128, 256], bf16, tag="esb")
                nc.scalar.activation(e_sb[:], sco[:], func=AF.Exp)
                a_sb = work.tile([128, 256], bf16, tag="asb")
                if "noz" in _ab:
                    nc.vector.tensor_copy(a_sb[:], e_sb[:])
                else:
                    zp = ps1.tile([128, 512], f32, tag="zpf", name="zp")
                    nc.tensor.matmul(zp[0:32, 0:256], mask_sb[:], e_sb[:],
                                     start=True, stop=True)
                    zr = work.tile([32, 256], bf16, tag="zr")
                    nc.vector.memset(zr[:], 0.0)
                    with nc.allow_low_precision(reason="1/Z in bf16: 0.4% on softmax scale"):
                        nc.vector.reciprocal(zr[0:8, :], zp[0:8, 0:256])
                    nc.tensor.matmul(zp[:, 256:512], maskT_sb[:], zr[:],
                                     start=True, stop=True)
                    nc.vector.tensor_mul(a_sb[:], e_sb[:], zp[:, 256:512])
                # ---- A2: bands stacked at partition base 0 ----
                a2 = work.tile([32, 1024], bf16, tag="a2")
                for u in range(4):
                    nc.vector.tensor_copy(a2[:, 256 * u:256 * (u + 1)],
                                          a_sb[32 * u:32 * (u + 1), :])
                # ---- AV: 64 pair matmuls ----
                avp = ps2.tile([128, 1024], f32, tag="avp", name="avp")
                if "noav" in _ab:
                    for q in range(4):
                        nc.scalar.activation(avp[:, 256 * q:256 * (q + 1)], a_sb[:],
                                             func=AF.Copy)
                else:
                    for g in range(64):
                        u, gg = g % 4, g // 4
                        nc.tensor.matmul(
                            avp[:, 16 * g:16 * (g + 1)],
                            vblk[:, 128 * g:128 * (g + 1)],
                            a2[:, 256 * u + 16 * gg:256 * u + 16 * (gg + 1)],
                            start=True, stop=True)
                # ---- evac to out2T chunks ----
                for cp in range(8):
                    nc.scalar.activation(
                        out2t[cp][:, 128 * sub:128 * (sub + 1)],
                        avp[:].rearrange("P (G cp i) -> P cp G i", G=8, cp=8)[:, cp],
                        func=AF.Copy)

            # ---- final projection for this group's s' columns ----
            _emit_final(sg)
    return nc


def _host_prep(x, Wq, bq, Wk, bk, Wv, bv, Wo, bo):
    """Returns per-core input maps."""
    xf = np.ascontiguousarray(x.reshape(NB * SB, EB))
    WqT = np.ascontiguousarray((Wq / 8.0).T).astype(ml_dtypes.bfloat16)
    WkT = np.ascontiguousarray(Wk.T).astype(ml_dtypes.bfloat16)
    WvT = np.ascontiguousarray(Wv.T).astype(ml_dtypes.bfloat16)
    WoPT = np.zeros((1024, 1024), np.float32)
    for cp in range(8):
        for a in range(2):
            w = 2 * cp + a
            WoPT[128 * cp + 64 * a:128 * cp + 64 * a + 64, :] = Wo[:, 64 * w:64 * (w + 1)].T
    WoPT = WoPT.astype(ml_dtypes.bfloat16)
    bias = np.zeros((128, 1048), np.float32)
    bias[:, 0:8] = (bq / 8.0).reshape(8, 128).T
    bias[:, 8:16] = bk.reshape(8, 128).T
    bias[:, 16:24] = bo.reshape(8, 128).T
    bias[:, 24:1048] = np.tile(bv[None, :], (128, 1))
    MASK = np.zeros((128, 32), np.float32)
    for u in range(4):
        for p in range(2):
            MASK[32 * u + 16 * p:32 * u + 16 * (p + 1), 2 * u + p] = 1.0
    MASKb = MASK.astype(ml_dtypes.bfloat16)
    MASKTb = np.ascontiguousarray(MASK.T).astype(ml_dtypes.bfloat16)
    in_maps = []
    for core in range(8):
        n, s0 = core // 2, (core % 2) * ROWS
        xs = xf[n * SB + s0:n * SB + s0 + ROWS]
        xT = np.ascontiguousarray(xs.T).astype(ml_dtypes.bfloat16)
        in_maps.append({"xt": xT, "wqt": WqT, "wkt": WkT, "wvt": WvT,
                        "wot": WoPT, "bias": bias, "mask": MASKb, "maskt": MASKTb})
    return in_maps


def _gather_out(core_outs):
    """core_outs: list of 8 per-core 'out' arrays (1024, 2048) -> full (N,S,E).
    out col = 16*G + i -> row i*256 + s0/16 + G."""
    out = np.zeros((NB, SB, EB), np.float32)
    cols = np.arange(ROWS)
    G, i = cols // 16, cols % 16
    for core in range(8):
        n, s0 = core // 2, (core % 2) * ROWS
        fT = np.asarray(core_outs[core])  # (1024, 2048)
        rows = i * 256 + (s0 // 16 + G)
        out[n, rows, :] = fT.T
    return out


def kernel(x, Wq, bq, Wk, bk, Wv, bv, Wo, bo):
    _install_birpatch()
    from concourse.bass_utils import run_bass_kernel_spmd

    if "nc" not in _CACHE:
        _CACHE["nc"] = _build_bass()
    nc = _CACHE["nc"]
    in_maps = _host_prep(np.asarray(x, np.float32), *[np.asarray(a, np.float32)
                         for a in (Wq, bq, Wk, bk, Wv, bv, Wo, bo)])
    res = run_bass_kernel_spmd(nc, in_maps, list(range(8)))
    return _gather_out([res.results[core]["out"] for core in range(8)])


# revision 31
# speedup vs baseline: 135.8198x; 1.0173x over previous
"""Trainium2 Bass kernel for nn_MultiHeadSelfAttention_62646392979761.

Math (per the buggy-einsum reference): per position s, heads attend to heads:
  Q,K,V = x@W{q,k,v}.T + b  (N,S,H,D);  scores[s] = Q[s]K[s]^T/8 (16x16);
  A = softmax_j;  AV[s] = A[s]V[s];  out2 = scrambled flat reshape;
  final = out2@Wo.T + bo.

Sharding: 8 cores x 2048 rows of the flattened (16384, 1024) x. Attention is
position-local; the scramble groups 16 consecutive positions, which never
cross a 2048-row shard. Zero cross-core communication.

Per-core pipeline (positions in 4 groups of 512, each 4 subtiles of 128).
Indices: position s = 128*sub + 16*G + w, pair-half p = w%2, c' = w//2%8,
pair g = 8*G + c' (u = g%4, gg = g//4), head i = 2c+a.
  1. QT/KT projections transposed (stationary = weight chunk, moving = xT):
     psum [128 f=(a,d), 512 s] per f-chunk; DVE bias evac -> qt/kt [(a,d),(c,s)].
  2. V projection natural (stationary = xT chunk): vn [128 s, 1024 (j,d)].
  3. QSTK [(p,d), 16g+i] / KBLK [(p,d), 32g+16p+j block-diag] via 4 strided
     DVE copies each from qt/kt. VBLK [(p,j), 128g+64p+d block-diag] via 2
     SBUF->SBUF gather DMAs from vn. Structural zeros memset on first use.
  4. Scores^T: 64 pair matmuls (k=128, m=32 col-rotated, n=16, stationary
     KBLK slab, moving QSTK slice) -> sco psum [(u,p,j), (gg,i)].
  5. Softmax: ACT exp -> E bf16; Z = mask^T-matmul (sums j over partitions);
     DVE reciprocal; Zb = maskT-matmul broadcast; A = E*Zb (DVE).
  6. AV: 64 pair matmuls (k=32 row-rotated, m=128, n=16, stationary VBLK
     slab, moving A slice) -> avp psum [(p,d), 16g+i].
  7. ACT copies avp -> out2T chunks [128 (a,d), 2048 (16G+i)] (c' = chunk).
  8. Final projection: stationary WoPT chunks, moving out2T -> psum
     [128 f, 512 s'], + bo (DVE) -> out_d (1024, 2048) f32.
Host: transposes x/weights, permutes Wo rows, post-scatters out columns
(col = 16G + i -> row i*256 + s0/16 + G).
"""

import math
import numpy as np
import ml_dtypes

ROWS = 2048
NB, SB, EB, HB, DB = 4, 4096, 1024, 16, 64

_CACHE = {}


def _split_waits_json(bir_bytes):
    """This env's walrus accepts only ONE embedded sync-wait per TPB
    instruction (NEURON_ISA_TPB_EVENTS has a single wait slot) but Tile emits
    several. Split excess on_wait entries onto standalone EventSemaphore
    instructions inserted just before, on the same engine — semantically
    identical on in-order engine queues."""
    import json
    d = json.loads(bir_bytes)
    for fn in d.get('functions', []):
        for bb in (fn.get('basic_blocks') or fn.get('blocks') or []):
            out = []
            for inst in bb.get('instructions', []):
                si = inst.get('sync_info')
                w = (si or {}).get('on_wait') or []
                if len(w) > 1:
                    for k, extra in enumerate(w[:-1]):
                        out.append({
                            'debug': inst.get('debug', 0),
                            'engine': inst['engine'],
                            'ins': [], 'outs': [],
                            'name': f"{inst['name']}-sw{k}",
                            'opcode': 'EventSemaphore',
                            'sync_info': {'on_wait': [extra], 'on_update': []},
                        })
                    si['on_wait'] = [w[-1]]
                out.append(inst)
            bb['instructions'] = out
    return json.dumps(d).encode()


def _install_birpatch():
    import concourse.bass_utils as bu
    import concourse.bass2jax as b2j
    if getattr(bu.compile_bir_kernel, '_waitsplit', False):
        return
    orig = bu.compile_bir_kernel

    def patched(bir_json, tmpdir, neff_name="file.neff"):
        return orig(_split_waits_json(bir_json), tmpdir, neff_name)

    patched._waitsplit = True
    bu.compile_bir_kernel = patched
    b2j.compile_bir_kernel = patched


def _build_bass(reps=1):
    import os
    import concourse.bass as bass
    import concourse.tile as tile
    from concourse import mybir
    _ab = set(os.environ.get("KABLATE", "").split(","))

    bf16 = mybir.dt.bfloat16
    f32 = mybir.dt.float32
    AF = mybir.ActivationFunctionType

    nc = bass.Bass(trn_type="TRN2")
    xt_d = nc.declare_dram_parameter("xt", [1024, ROWS], bf16, isOutput=False)
    wqT_d = nc.declare_dram_parameter("wqt", [1024, 1024], bf16, isOutput=False)
    wkT_d = nc.declare_dram_parameter("wkt", [1024, 1024], bf16, isOutput=False)
    wvT_d = nc.declare_dram_parameter("wvt", [1024, 1024], bf16, isOutput=False)
    woT_d = nc.declare_dram_parameter("wot", [1024, 1024], bf16, isOutput=False)
    bias_d = nc.declare_dram_parameter("bias", [128, 1048], f32, isOutput=False)
    mask_d = nc.declare_dram_parameter("mask", [128, 32], bf16, isOutput=False)
    maskT_d = nc.declare_dram_parameter("maskt", [32, 128], bf16, isOutput=False)
    out_d = nc.declare_dram_parameter("out", [1024, ROWS], f32, isOutput=True)

    from contextlib import ExitStack
    with ExitStack() as ctx:
        tc = ctx.enter_context(tile.TileContext(nc))
        const = ctx.enter_context(tc.tile_pool(name="const", bufs=1))
        work = ctx.enter_context(tc.tile_pool(name="work", bufs=2))
        vnp = ctx.enter_context(tc.tile_pool(name="vnp", bufs=2))
        ppj = ctx.enter_context(tc.tile_pool(name="ppj", bufs=2, space="PSUM"))
        pvj = ctx.enter_context(tc.tile_pool(name="pvj", bufs=1, space="PSUM"))
        psc = ctx.enter_context(tc.tile_pool(name="psc", bufs=2, space="PSUM"))
        ps1 = ctx.enter_context(tc.tile_pool(name="ps1", bufs=1, space="PSUM"))
        drp = ctx.enter_context(tc.tile_pool(name="drp", bufs=4, space="DRAM"))

        if reps != 1:
            ctx.enter_context(tc.For_i(0, reps))

        # ---- persistent tensors (reloaded every rep for honest timing) ----
        wq_sb = const.tile([128, 8192], bf16, tag="wq")
        wk_sb = const.tile([128, 8192], bf16, tag="wk")
        wv_sb = const.tile([128, 8192], bf16, tag="wv")
        wo_sb = const.tile([128, 8192], bf16, tag="wo")
        bias_sb = const.tile([128, 1048], f32, tag="bias")
        mask_sb = const.tile([128, 32], bf16, tag="mask")
        maskT_sb = const.tile([32, 128], bf16, tag="maskt")
        out2t = [const.tile([128, 2048], bf16, tag=f"o2t{c}", name=f"o2t{c}")
                 for c in range(8)]
        kblks = [const.tile([128, 2048], bf16, tag=f"kblk{b}", name=f"kblk{b}")
                 for b in range(2)]
        vblks = [const.tile([32, 8192], bf16, tag=f"vblk{b}", name=f"vblk{b}")
                 for b in range(2)]
        for b in range(2):
            nc.gpsimd.memset(kblks[b][:], 0.0)
            nc.gpsimd.memset(vblks[b][:], 0.0)

        # weight layout: sb[p, 1024*ce + 128*cf + m] = W_T[128*ce + p, 128*cf + m]
        for sb, d in ((wq_sb, wqT_d), (wk_sb, wkT_d), (wv_sb, wvT_d), (wo_sb, woT_d)):
            nc.sync.dma_start(
                sb[:].rearrange("p (ce f) -> p ce f", ce=8),
                d[:].rearrange("(ce p) f -> p ce f", ce=8),
            )
        nc.sync.dma_start(bias_sb[:], bias_d[:])
        nc.sync.dma_start(mask_sb[:], mask_d[:])
        nc.sync.dma_start(maskT_sb[:], maskT_d[:])
        bqT = bias_sb[:, 0:8]
        bkT = bias_sb[:, 8:16]
        boT = bias_sb[:, 16:24]
        bvR = bias_sb[:, 24:1048]

        def _emit_final(fg):
            for cf in range(8):
                pf = ps1.tile([128, 512], f32, tag="zpf", name="pf")
                for cp in range(8):
                    nc.tensor.matmul(
                        pf[:], wo_sb[:, 1024 * cp + 128 * cf:1024 * cp + 128 * (cf + 1)],
                        out2t[cp][:, 512 * fg:512 * (fg + 1)],
                        start=(cp == 0), stop=(cp == 7))
                osb = work.tile([128, 512], f32, tag="osb")
                nc.vector.tensor_scalar_add(osb[:], pf[:], boT[:, cf:cf + 1])
                nc.sync.dma_start(out_d[128 * cf:128 * (cf + 1), 512 * fg:512 * (fg + 1)], osb[:])

        for sg in range(4):
            # ---- load xT group: [128, (ce, s 512)] ----
            xt_sb = work.tile([128, 4096], bf16, tag="xt")
            nc.sync.dma_start(
                xt_sb[:].rearrange("p (ce s) -> p ce s", ce=8),
                xt_d[:].rearrange("(ce p) s -> p ce s", ce=8)[:, :, 512 * sg:512 * (sg + 1)],
            )
            # ---- QT / KT projections (transposed: stationary = weights) ----
            qt_sg = work.tile([128, 4096], bf16, tag="qt")
            kt_sg = work.tile([128, 4096], bf16, tag="kt")
            for w_sb, bT, dst in ((wq_sb, bqT, qt_sg), (wk_sb, bkT, kt_sg)):
                for cf in range(8):
                    pp = ppj.tile([128, 512], f32, tag="pp", name="pp")
                    for ce in range(8):
                        nc.tensor.matmul(
                            pp[:], w_sb[:, 1024 * ce + 128 * cf:1024 * ce + 128 * (cf + 1)],
                            xt_sb[:, 512 * ce:512 * (ce + 1)],
                            start=(ce == 0), stop=(ce == 7))
                    nc.vector.tensor_scalar_add(
                        dst[:, 512 * cf:512 * (cf + 1)], pp[:], bT[:, cf:cf + 1])
            # ---- V projection (natural: stationary = xT chunk) ----
            vns = []
            for sl in range(4):
                vn = vnp.tile([128, 1024], bf16, tag="vn")
                for h in range(2):
                    pv = pvj.tile([128, 512], f32, tag="pv", name="pv")
                    for ce in range(8):
                        nc.tensor.matmul(
                            pv[:],
                            xt_sb[:, 512 * ce + 128 * sl:512 * ce + 128 * (sl + 1)],
                            wv_sb[:, 1024 * ce + 512 * h:1024 * ce + 512 * (h + 1)],
                            start=(ce == 0), stop=(ce == 7))
                    nc.vector.tensor_add(vn[:, 512 * h:512 * (h + 1)], pv[:],
                                         bvR[:, 512 * h:512 * (h + 1)])
                vdr = drp.tile([128, 1024], bf16, tag="vdr")
                nc.sync.dma_start(vdr[:], vn[:])
                vns.append(vdr)

            for sl in range(4):
                sub = 4 * sg + sl
                vdr = vns[sl]
                # ---- VBLK [32 (p,j), 128g+64p+d] via 2 gather DMAs (DRAM src) ----
                vblk = vblks[sub % 2]
                for p in range(2 if "novblk" not in _ab else 0):
                    nc.sync.dma_start(
                        vblk[:][16 * p:16 * (p + 1), :].rearrange(
                            "P (g two d) -> P g two d", g=64, two=2)[:, :, p, :],
                        vdr[:].rearrange("(G cp t) (j d) -> t j G cp d",
                                         G=8, cp=8, t=2, j=16)[p],
                    )
                # ---- QSTK / KBLK via 4 strided DVE copies each ----
                qstk = work.tile([128, 1024], bf16, tag="qstk", name="qstk")
                kblk = kblks[sub % 2]
                for p in range(2):
                    for a in range(2):
                        src_q = qt_sg[:][64 * a:64 * (a + 1), :].rearrange(
                            "P (c sl G cp t) -> P sl t G cp c",
                            c=8, sl=4, G=8, cp=8, t=2)[:, sl, p]
                        dst_q = qstk[:][64 * p:64 * (p + 1), :].rearrange(
                            "P (G cp c t) -> P t G cp c", G=8, cp=8, c=8, t=2)[:, a]
                        nc.vector.tensor_copy(dst_q, src_q)
                        src_k = kt_sg[:][64 * a:64 * (a + 1), :].rearrange(
                            "P (c sl G cp t) -> P sl t G cp c",
                            c=8, sl=4, G=8, cp=8, t=2)[:, sl, p]
                        dst_k = kblk[:][64 * p:64 * (p + 1), :].rearrange(
                            "P (G cp two c t) -> P two t G cp c",
                            G=8, cp=8, two=2, c=8, t=2)[:, p, a]
                        nc.vector.tensor_copy(dst_k, src_k)
                # ---- scores^T: 64 pair matmuls ----
                sco = psc.tile([128, 256], f32, tag="sco", name="sco")
                if "nosco" in _ab:
                    nc.scalar.activation(sco[:], qstk[:, 0:256], func=AF.Copy)
                else:
                    for g in range(64):
                        u, gg = g % 4, g // 4
                        nc.tensor.matmul(
                            sco[32 * u:32 * (u + 1), 16 * gg:16 * (gg + 1)],
                            kblk[:, 32 * g:32 * (g + 1)],
                            qstk[:, 16 * g:16 * (g + 1)],
                            start=True, stop=True, tile_position=(0, 32 * u))
                # ---- softmax ----
                e_sb = work.tile([128, 256], bf16, tag="esb")
                nc.scalar.activation(e_sb[:], sco[:], func=AF.Exp)
                a_sb = work.tile([128, 256], bf16, tag="asb")
                if "noz" in _ab:
                    nc.vector.tensor_copy(a_sb[:], e_sb[:])
                else:
                    zp = ps1.tile([128, 512], f32, tag="zpf", name="zp")
                    nc.tensor.matmul(zp[0:32, 0:256], mask_sb[:], e_sb[:],
                                     start=True, stop=True)
                    zr = work.tile([32, 256], bf16, tag="zr")
                    nc.vector.memset(zr[:], 0.0)
                    with nc.allow_low_precision(reason="1/Z in bf16: 0.4% on softmax scale"):
                        nc.vector.reciprocal(zr[0:8, :], zp[0:8, 0:256])
                    nc.tensor.matmul(zp[:, 256:512], maskT_sb[:], zr[:],
                                     start=True, stop=True)
                    nc.vector.tensor_mul(a_sb[:], e_sb[:], zp[:, 256:512])
                # ---- A2: bands stacked at partition base 0 ----
                a2 = work.tile([32, 1024], bf16, tag="a2")
                for u in range(4):
                    nc.vector.tensor_copy(a2[:, 256 * u:256 * (u + 1)],
                                          a_sb[32 * u:32 * (u + 1), :])
                # ---- AV: 64 pair matmuls ----
                avp = ps1.tile([128, 1024], f32, tag="avp", name="avp")
                if "noav" in _ab:
                    for q in range(4):
                        nc.scalar.activation(avp[:, 256 * q:256 * (q + 1)], a_sb[:],
                                             func=AF.Copy)
                else:
                    for g in range(64):
                        u, gg = g % 4, g // 4
                        nc.tensor.matmul(
                            avp[:, 16 * g:16 * (g + 1)],
                            vblk[:, 128 * g:128 * (g + 1)],
                            a2[:, 256 * u + 16 * gg:256 * u + 16 * (gg + 1)],
                            start=True, stop=True)
                # ---- evac to out2T chunks ----
                for cp in range(8):
                    nc.scalar.activation(
                        out2t[cp][:, 128 * sub:128 * (sub + 1)],
                        avp[:].rearrange("P (G cp i) -> P cp G i", G=8, cp=8)[:, cp],
                        func=AF.Copy)

            # ---- final projection for this group's s' columns ----
            _emit_final(sg)
    return nc


def _host_prep(x, Wq, bq, Wk, bk, Wv, bv, Wo, bo):
    """Returns per-core input maps."""
    xf = np.ascontiguousarray(x.reshape(NB * SB, EB))
    WqT = np.ascontiguousarray((Wq / 8.0).T).astype(ml_dtypes.bfloat16)
    WkT = np.ascontiguousarray(Wk.T).astype(ml_dtypes.bfloat16)
    WvT = np.ascontiguousarray(Wv.T).astype(ml_dtypes.bfloat16)
    WoPT = np.zeros((1024, 1024), np.float32)
    for cp in range(8):
        for a in range(2):
            w = 2 * cp + a
            WoPT[128 * cp + 64 * a:128 * cp + 64 * a + 64, :] = Wo[:, 64 * w:64 * (w + 1)].T
    WoPT = WoPT.astype(ml_dtypes.bfloat16)
    bias = np.zeros((128, 1048), np.float32)
    bias[:, 0:8] = (bq / 8.0).reshape(8, 128).T
    bias[:, 8:16] = bk.reshape(8, 128).T
    bias[:, 16:24] = bo.reshape(8, 128).T
    bias[:, 24:1048] = np.tile(bv[None, :], (128, 1))
    MASK = np.zeros((128, 32), np.float32)
    for u in range(4):
        for p in range(2):
            MASK[32 * u + 16 * p:32 * u + 16 * (p + 1), 2 * u + p] = 1.0
    MASKb = MASK.astype(ml_dtypes.bfloat16)
    MASKTb = np.ascontiguousarray(MASK.T).astype(ml_dtypes.bfloat16)
    in_maps = []
    for core in range(8):
        n, s0 = core // 2, (core % 2) * ROWS
        xs = xf[n * SB + s0:n * SB + s0 + ROWS]
        xT = np.ascontiguousarray(xs.T).astype(ml_dtypes.bfloat16)
        in_maps.append({"xt": xT, "wqt": WqT, "wkt": WkT, "wvt": WvT,
                        "wot": WoPT, "bias": bias, "mask": MASKb, "maskt": MASKTb})
    return in_maps


def _gather_out(core_outs):
    """core_outs: list of 8 per-core 'out' arrays (1024, 2048) -> full (N,S,E).
    out col = 16*G + i -> row i*256 + s0/16 + G."""
    out = np.zeros((NB, SB, EB), np.float32)
    cols = np.arange(ROWS)
    G, i = cols // 16, cols % 16
    for core in range(8):
        n, s0 = core // 2, (core % 2) * ROWS
        fT = np.asarray(core_outs[core])  # (1024, 2048)
        rows = i * 256 + (s0 // 16 + G)
        out[n, rows, :] = fT.T
    return out


def kernel(x, Wq, bq, Wk, bk, Wv, bv, Wo, bo):
    _install_birpatch()
    from concourse.bass_utils import run_bass_kernel_spmd

    if "nc" not in _CACHE:
        _CACHE["nc"] = _build_bass()
    nc = _CACHE["nc"]
    in_maps = _host_prep(np.asarray(x, np.float32), *[np.asarray(a, np.float32)
                         for a in (Wq, bq, Wk, bk, Wv, bv, Wo, bo)])
    res = run_bass_kernel_spmd(nc, in_maps, list(range(8)))
    return _gather_out([res.results[core]["out"] for core in range(8)])
